# revision 8
# baseline (speedup 1.0000x reference)
"""CTC loss kernel for Trainium2 (8 NeuronCores, data-parallel over batch).

Math: with raw logits G[b,t,s] = pred[b,t,ext[b,s]] (ext = blank-interleaved
targets) the CTC forward recursion commutes with the per-frame log-softmax
normalizer: running the recursion on raw logits and subtracting
sum_t logsumexp_c(pred[b,t,:]) at the end gives the same loss. The chip does
(1) sum_c exp(pred) per (b,t) via streaming ACT exp+accumulate (the
memory-bound bulk, ~68 MB/core at the SBUF-fabric ceiling) and (2) the
probability-space forward recursion on the VectorEngine.

The recursion step new[s] = p[s]*(A[s] + A[s-1] + sk[s]*A[s-2]) is linear in
A, so K=4 consecutive steps compose into one 9-tap banded matrix whose
coefficients depend only on p/sk — the host precomputes them (bf16, all
terms positive so errors stay relative). On-chip each fused step is ONE
windowed tensor_mul (overlapping-window AP, free dims [(1,51),(1,9)])
against the coefficient block plus ONE reduce_add: DVE cost follows
(N+151)/0.96ns, so 40 fused steps ≈ 58us of serial chain vs ~110us for
per-step evaluation, fully hidden under the stream. Renormalization (every
8 steps = every 2 fused, against overflow) records the reciprocal of the
running max and folds the multiply into the next fused step's
scalar_tensor_tensor; the host compensates with -log(rn) in float64.
"""

import sys

sys.path.insert(0, "/opt/trn_rl_repo")

import numpy as np

import bass_rust
import concourse.bacc as bacc
import concourse.tile as tile
from concourse import mybir
from concourse.bass_utils import run_bass_kernel_spmd

B, T, C, L = 128, 160, 6625, 25
S = 2 * L + 1  # 51 CTC states
KF = 4  # CTC steps fused per DVE step
WQ = 2 * KF + 1  # 9-tap window
GD = WQ - 1  # 8 guard columns
SG = S + GD  # state tile cols: guards + states
QF = S * WQ  # 459 coefficients per fused step
NSTEP = T - 1  # 159 raw steps
NFUSED = (NSTEP + KF - 1) // KF  # 40 fused steps (last covers 3 raw)
N_CORES = 8
BS = B // N_CORES  # 16 samples per core
TBLK = 8  # t-values per 128-row streaming block (8*16 = 128 rows)
NBLK = T // TBLK  # 20
# finer parts for the first/last streaming block: earlier pipeline start,
# smaller exposed tail.
QCHUNKS = [(0, 1657), (1657, 3313), (3313, 4969), (4969, 6625)]
NQCH = len(QCHUNKS)
QCHMAX = max(c1 - c0 for c0, c1 in QCHUNKS)
NEG = -1.0e4  # exp() underflows to exactly 0.0f
NREN = 19  # renorm after fused steps 1,3,...,37 (raw t = 8,16,...,152)

f32 = mybir.dt.float32
bf16 = mybir.dt.bfloat16
f16 = mybir.dt.float16
Exp = mybir.ActivationFunctionType.Exp

_CACHE = {}


def _win(ap, part_stride, n_part, s_stride):
    """Windowed view [n_part, S, WQ]: addr = offset + s*s_stride + d."""
    v = ap.copy()
    v.ap = bass_rust.VecI64Pair(
        [[part_stride, n_part], [s_stride, S], [1, WQ]])
    return v


def _build_program():
    if "nc" in _CACHE:
        return _CACHE["nc"]
    nc = bacc.Bacc("TRN2", target_bir_lowering=False, debug=False,
                   num_devices=N_CORES)
    pred_d = nc.dram_tensor("pred", [BS, T, C], f32, kind="ExternalInput").ap()
    q_d = nc.dram_tensor("q", [BS, NFUSED * QF], bf16,
                         kind="ExternalInput").ap()
    a0_d = nc.dram_tensor("a0", [BS, SG], f32, kind="ExternalInput").ap()
    acc_d = nc.dram_tensor("acc", [128, NBLK * NQCH], f32,
                           kind="ExternalOutput").ap()
    afin_d = nc.dram_tensor("afin", [BS, S], f32, kind="ExternalOutput").ap()
    rnorm_d = nc.dram_tensor("rnorm", [BS, NREN], f32,
                             kind="ExternalOutput").ap()

    with tile.TileContext(nc) as tc:
        with (
            tc.tile_pool(name="persist", bufs=1) as pp,
            tc.tile_pool(name="steps", bufs=2) as stepp,
            tc.tile_pool(name="stream", bufs=5) as spool,
        ):
            qt = pp.tile([BS, NFUSED * QF], bf16, tag="qt")
            Aa = pp.tile([BS, SG], f32, tag="Aa")
            Ab = pp.tile([BS, SG], f32, tag="Ab")
            Mt = pp.tile([BS, NREN], f32, tag="Mt")
            acc = pp.tile([128, NBLK * NQCH], f32, tag="acc")

            # acc zeroed once; middle blocks only write col j*4+0. Emitted
            # before any ACT accum write so the WAW order is correct.
            nc.vector.memset(acc[:], 0.0)
            nc.vector.memset(Ab[:, 0:GD], 0.0)

            # ---- recursion inputs first on the sync ring (the 16-partition
            # q transfer is SBUF-port-limited, so its first chunk goes ahead
            # of the wide stream chunks), then stream block 0 entirely on the
            # sync HWDGE ring as fp32 — it starts streaming ~5us before the
            # SWDGE Q7 pipeline warms up and no ACT dependency is parked
            # behind the slow SWDGE warmup.
            nc.sync.dma_start(out=Aa[:], in_=a0_d[:])
            qq = (NFUSED * QF) // 4
            nc.sync.dma_start(out=qt[:, 0:qq], in_=q_d[:, 0:qq])
            for ci, (c0, c1) in enumerate(QCHUNKS):
                w = c1 - c0
                cp = spool.tile([128, QCHMAX], f32, tag="part32")
                nc.sync.dma_start(out=cp[:, :w], in_=pred_d[:, 0:TBLK, c0:c1])
                nc.scalar.activation(cp[:, :w], cp[:, :w], Exp,
                                     accum_out=acc[:, ci:ci + 1])
                if ci < 3:
                    lo = (ci + 1) * qq
                    hi = NFUSED * QF if ci == 2 else lo + qq
                    nc.sync.dma_start(out=qt[:, lo:hi], in_=q_d[:, lo:hi])

            # ---- DVE-only fused forward recursion.
            cur, nxt = Aa, Ab
            k = 0
            pend = None  # per-partition scalar to multiply in (renorm fold)
            qstride = NFUSED * QF
            for tau in range(NFUSED):
                wtl = stepp.tile([BS, QF], f32, tag="w")
                av = _win(cur[:], SG, BS, 1)
                qv = _win(qt[:, tau * QF:(tau + 1) * QF], qstride, BS, WQ)
                wv = _win(wtl[:], QF, BS, WQ)
                if pend is None:
                    nc.vector.tensor_mul(out=wv, in0=av, in1=qv)
                else:
                    nc.vector.scalar_tensor_tensor(
                        out=wv, in0=av, scalar=pend, in1=qv,
                        op0=mybir.AluOpType.mult, op1=mybir.AluOpType.mult)
                    pend = None
                nc.vector.tensor_reduce(out=nxt[:, GD:GD + S], in_=wv,
                                        axis=mybir.AxisListType.X,
                                        op=mybir.AluOpType.add)
                if tau % 2 == 1 and k < NREN:
                    mx = stepp.tile([BS, 1], f32, tag="mx")
                    nc.vector.reduce_max(mx[:], nxt[:, GD:GD + S],
                                         axis=mybir.AxisListType.X)
                    # record the actual multiplier; host compensates -log(rn)
                    nc.vector.reciprocal(out=Mt[:, k:k + 1], in_=mx[:])
                    pend = Mt[:, k:k + 1]
                    k += 1
                cur, nxt = nxt, cur
            assert k == NREN
            nc.sync.dma_start(out=afin_d[:], in_=cur[:, GD:GD + S])
            nc.sync.dma_start(out=rnorm_d[:], in_=Mt[:])

            # ---- streaming sum(exp(pred)) over C, 128 (b,t) rows per block.
            # SWDGE inline fp32->fp16 cast halves SBUF-write traffic so the
            # HBM/fabric read side binds. Last block chunked for a shorter
            # exposed tail.
            for j in range(1, NBLK):
                src = pred_d[:, j * TBLK:(j + 1) * TBLK, :]
                if j == NBLK - 1:
                    for ci, (c0, c1) in enumerate(QCHUNKS):
                        w = c1 - c0
                        cp = spool.tile([128, QCHMAX], f16, tag="chunkpart")
                        nc.gpsimd.dma_start(out=cp[:, :w],
                                            in_=src[:, :, c0:c1])
                        nc.scalar.activation(
                            cp[:, :w], cp[:, :w], Exp,
                            accum_out=acc[:, j * NQCH + ci:j * NQCH + ci + 1])
                else:
                    ct = spool.tile([128, C], f16, tag="chunk")
                    nc.gpsimd.dma_start(out=ct[:], in_=src)
                    nc.scalar.activation(
                        ct[:], ct[:], Exp,
                        accum_out=acc[:, j * NQCH:j * NQCH + 1])
            nc.sync.dma_start(out=acc_d[:], in_=acc[:])

    nc.compile()
    _CACHE["nc"] = nc
    return nc


def _compose_bands(P, sk):
    """Fuse per-step band matrices into KF-step 9-tap coefficient blocks.

    P: [B, T, S] step probabilities (raw-logit exp, masked states = 0)
    sk: [B, S] skip-transition mask
    Returns Q [B, NFUSED, S, WQ] with Q[..., s, d] = coeff of A_old[s-(GD-d)].
    """
    b1 = P.copy()  # M[s, s-1] coeff, invalid at s=0
    b1[:, :, 0] = 0.0
    b2 = P * sk[:, None, :]  # M[s, s-2] coeff, invalid at s<2
    b2[:, :, :2] = 0.0
    Q = np.zeros((B, NFUSED, S, WQ), dtype=np.float64)
    for tau in range(NFUSED):
        t0 = 1 + tau * KF
        nk = min(KF, T - t0)
        # bands C[o][s] = coeff of A_old[s-o]; start with identity
        Cb = {0: np.ones((B, S), dtype=np.float64)}
        for i in range(nk):
            t = t0 + i
            Mb = {0: P[:, t].astype(np.float64),
                  1: b1[:, t].astype(np.float64),
                  2: b2[:, t].astype(np.float64)}
            Nb = {}
            for o2, m in Mb.items():
                for oc, cvec in Cb.items():
                    o = o2 + oc
                    sh = np.zeros((B, S), dtype=np.float64)
                    sh[:, o2:] = cvec[:, :S - o2] if o2 else cvec
                    term = m * sh
                    if o in Nb:
                        Nb[o] += term
                    else:
                        Nb[o] = term
            Cb = Nb
        for o, cvec in Cb.items():
            Q[:, tau, :, GD - o] = cvec
    return Q


def prepare_in_maps(pred, targets, lens):
    """Host prep: extended labels, gathered probs, fused band coefficients."""
    ext = np.zeros((B, S), dtype=np.int64)
    ext[:, 1::2] = targets
    G = pred[np.arange(B)[:, None, None], np.arange(T)[None, :, None],
             ext[:, None, :]]  # [B, T, S]
    valid = np.arange(S)[None, :] < (2 * lens + 1)[:, None]  # [B, S]
    G = np.where(valid[:, None, :], G, NEG)
    P = np.exp(G.astype(np.float64)).astype(np.float32)  # [B, T, S]
    sk = np.pad((ext[:, 2:] != ext[:, :-2]) & (ext[:, 2:] != 0),
                ((0, 0), (2, 0))).astype(np.float32)  # [B, S]
    Q = _compose_bands(P, sk).astype(np.float32)
    Qb = Q.astype(mybir.dt.np(bf16))
    a0 = np.zeros((B, SG), dtype=np.float32)
    a0[:, GD:GD + 2] = P[:, 0, 0:2]
    in_maps = []
    for c in range(N_CORES):
        sl = slice(c * BS, (c + 1) * BS)
        in_maps.append({
            "pred": np.ascontiguousarray(pred[sl]),
            "q": np.ascontiguousarray(Qb[sl].reshape(BS, NFUSED * QF)),
            "a0": np.ascontiguousarray(a0[sl]),
        })
    return in_maps


def finish_host(results, lens):
    """Combine per-core outputs into the scalar mean loss (float64)."""
    loss_b = np.zeros(B, dtype=np.float64)
    with np.errstate(divide="ignore", invalid="ignore"):
        for c in range(N_CORES):
            r = results[c]
            acc = r["acc"].astype(np.float64)  # [128, NBLK*NQCH]
            lse = np.log(acc.reshape(128, NBLK, NQCH).sum(-1))  # [128, NBLK]
            # row p = b*TBLK + t_off; t = j*TBLK + t_off
            s_lse = lse.reshape(BS, TBLK, NBLK).sum((1, 2))  # [BS]
            afin = r["afin"].astype(np.float64)  # [BS, S]
            rn = r["rnorm"].astype(np.float64)  # [BS, NREN] 1/max multipliers
            log_carry = np.log(rn).sum(1)  # [BS]
            for b in range(BS):
                gb = c * BS + b
                sE = 2 * int(lens[gb])
                le = np.logaddexp(np.log(afin[b, sE]), np.log(afin[b, sE - 1]))
                loss_b[gb] = s_lse[b] + log_carry[b] - le
    loss_b = np.where(loss_b >= 1e29, 0.0, loss_b)
    loss_b = np.where(np.isfinite(loss_b), loss_b, 0.0)
    loss = np.mean(loss_b / np.maximum(lens.astype(np.float64), 1.0))
    return np.float32(loss)


def kernel(pred, targets, targets_lengths):
    pred = np.asarray(pred, dtype=np.float32)
    targets = np.asarray(targets).astype(np.int64)
    lens = np.asarray(targets_lengths).astype(np.int64)

    nc = _build_program()
    in_maps = prepare_in_maps(pred, targets, lens)
    res = run_bass_kernel_spmd(nc, in_maps, core_ids=list(range(N_CORES)))
    return finish_host(res.results, lens)


# revision 9
# speedup vs baseline: 1.1660x; 1.1660x over previous
"""CTC loss kernel for Trainium2 (8 NeuronCores, data-parallel over batch).

Math: with raw logits G[b,t,s] = pred[b,t,ext[b,s]] (ext = blank-interleaved
targets) the CTC forward recursion commutes with the per-frame log-softmax
normalizer: running the recursion on raw logits and subtracting
sum_t logsumexp_c(pred[b,t,:]) at the end gives the same loss. The chip does
(1) sum_c exp(pred) per (b,t) via streaming ACT exp+accumulate (the
memory-bound bulk, ~68 MB/core at the SBUF-fabric ceiling) and (2) the
probability-space forward recursion on the VectorEngine.

The recursion step new[s] = p[s]*(A[s] + A[s-1] + sk[s]*A[s-2]) is linear in
A, so K=4 consecutive steps compose into one 9-tap banded matrix whose
coefficients depend only on p/sk — the host precomputes them (bf16, all
terms positive so errors stay relative). On-chip each fused step is ONE
windowed tensor_mul (overlapping-window AP, free dims [(1,51),(1,9)])
against the coefficient block plus ONE reduce_add: DVE cost follows
(N+151)/0.96ns, so 40 fused steps ≈ 58us of serial chain vs ~110us for
per-step evaluation, fully hidden under the stream. Renormalization (every
8 steps = every 2 fused, against overflow) records the reciprocal of the
running max and folds the multiply into the next fused step's
scalar_tensor_tensor; the host compensates with -log(rn) in float64.
"""

import sys

sys.path.insert(0, "/opt/trn_rl_repo")

import numpy as np

import bass_rust
import concourse.bacc as bacc
import concourse.tile as tile
from concourse import mybir
from concourse.bass_utils import run_bass_kernel_spmd

B, T, C, L = 128, 160, 6625, 25
S = 2 * L + 1  # 51 CTC states
KF = 4  # CTC steps fused per DVE step
WQ = 2 * KF + 1  # 9-tap window
GD = WQ - 1  # 8 guard columns
SG = S + GD  # state tile cols: guards + states
QF = S * WQ  # 459 coefficients per fused step
NSTEP = T - 1  # 159 raw steps
NFUSED = (NSTEP + KF - 1) // KF  # 40 fused steps (last covers 3 raw)
N_CORES = 8
BS = B // N_CORES  # 16 samples per core
TBLK = 8  # t-values per 128-row streaming block (8*16 = 128 rows)
NBLK = T // TBLK  # 20
# finer parts for the first/last streaming block: earlier pipeline start,
# smaller exposed tail.
QCHUNKS = [(0, 1657), (1657, 3313), (3313, 4969), (4969, 6625)]
NQCH = len(QCHUNKS)
QCHMAX = max(c1 - c0 for c0, c1 in QCHUNKS)
NEG = -1.0e4  # exp() underflows to exactly 0.0f
NREN = 19  # renorm after fused steps 1,3,...,37 (raw t = 8,16,...,152)

f32 = mybir.dt.float32
bf16 = mybir.dt.bfloat16
f16 = mybir.dt.float16
Exp = mybir.ActivationFunctionType.Exp

_CACHE = {}


def _win(ap, part_stride, n_part, s_stride):
    """Windowed view [n_part, S, WQ]: addr = offset + s*s_stride + d."""
    v = ap.copy()
    v.ap = bass_rust.VecI64Pair(
        [[part_stride, n_part], [s_stride, S], [1, WQ]])
    return v


def _build_program():
    if "nc" in _CACHE:
        return _CACHE["nc"]
    nc = bacc.Bacc("TRN2", target_bir_lowering=False, debug=False,
                   num_devices=N_CORES)
    pred_d = nc.dram_tensor("pred", [BS, T, C], f32, kind="ExternalInput").ap()
    q_d = nc.dram_tensor("q", [BS, NFUSED * QF], bf16,
                         kind="ExternalInput").ap()
    a0_d = nc.dram_tensor("a0", [BS, SG], f32, kind="ExternalInput").ap()
    acc_d = nc.dram_tensor("acc", [128, NBLK * NQCH], f32,
                           kind="ExternalOutput").ap()
    afin_d = nc.dram_tensor("afin", [BS, S], f32, kind="ExternalOutput").ap()
    rnorm_d = nc.dram_tensor("rnorm", [BS, NREN], f32,
                             kind="ExternalOutput").ap()

    with tile.TileContext(nc) as tc:
        with (
            tc.tile_pool(name="persist", bufs=1) as pp,
            tc.tile_pool(name="steps", bufs=2) as stepp,
            tc.tile_pool(name="stream", bufs=5) as spool,
        ):
            qt = pp.tile([BS, NFUSED * QF], bf16, tag="qt")
            Aa = pp.tile([BS, SG], f32, tag="Aa")
            Ab = pp.tile([BS, SG], f32, tag="Ab")
            Mt = pp.tile([BS, NREN], f32, tag="Mt")
            acc = pp.tile([128, NBLK * NQCH], f32, tag="acc")

            # acc zeroed once; middle blocks only write col j*4+0. Emitted
            # before any ACT accum write so the WAW order is correct.
            nc.vector.memset(acc[:], 0.0)
            nc.vector.memset(Ab[:, 0:GD], 0.0)

            # ---- recursion inputs first on the sync ring (the 16-partition
            # q transfer is SBUF-port-limited, so its first chunk goes ahead
            # of the wide stream chunks), then stream block 0 entirely on the
            # sync HWDGE ring as fp32 — it starts streaming ~5us before the
            # SWDGE Q7 pipeline warms up and no ACT dependency is parked
            # behind the slow SWDGE warmup.
            nc.sync.dma_start(out=Aa[:], in_=a0_d[:])
            qq = (NFUSED * QF) // 4
            for ci, (c0, c1) in enumerate(QCHUNKS):
                w = c1 - c0
                cp = spool.tile([128, QCHMAX], f32, tag="part32")
                nc.sync.dma_start(out=cp[:, :w], in_=pred_d[:, 0:TBLK, c0:c1])
                nc.scalar.activation(cp[:, :w], cp[:, :w], Exp,
                                     accum_out=acc[:, ci:ci + 1])
                lo = ci * qq
                hi = NFUSED * QF if ci == 3 else lo + qq
                nc.sync.dma_start(out=qt[:, lo:hi], in_=q_d[:, lo:hi])

            # ---- DVE-only fused forward recursion.
            cur, nxt = Aa, Ab
            k = 0
            pend = None  # per-partition scalar to multiply in (renorm fold)
            qstride = NFUSED * QF
            for tau in range(NFUSED):
                wtl = stepp.tile([BS, QF], f32, tag="w")
                av = _win(cur[:], SG, BS, 1)
                qv = _win(qt[:, tau * QF:(tau + 1) * QF], qstride, BS, WQ)
                wv = _win(wtl[:], QF, BS, WQ)
                if pend is None:
                    nc.vector.tensor_mul(out=wv, in0=av, in1=qv)
                else:
                    nc.vector.scalar_tensor_tensor(
                        out=wv, in0=av, scalar=pend, in1=qv,
                        op0=mybir.AluOpType.mult, op1=mybir.AluOpType.mult)
                    pend = None
                nc.vector.tensor_reduce(out=nxt[:, GD:GD + S], in_=wv,
                                        axis=mybir.AxisListType.X,
                                        op=mybir.AluOpType.add)
                if tau % 2 == 1 and k < NREN:
                    mx = stepp.tile([BS, 1], f32, tag="mx")
                    nc.vector.reduce_max(mx[:], nxt[:, GD:GD + S],
                                         axis=mybir.AxisListType.X)
                    # record the actual multiplier; host compensates -log(rn)
                    nc.vector.reciprocal(out=Mt[:, k:k + 1], in_=mx[:])
                    pend = Mt[:, k:k + 1]
                    k += 1
                cur, nxt = nxt, cur
            assert k == NREN
            nc.sync.dma_start(out=afin_d[:], in_=cur[:, GD:GD + S])
            nc.sync.dma_start(out=rnorm_d[:], in_=Mt[:])

            # ---- streaming sum(exp(pred)) over C, 128 (b,t) rows per block.
            # SWDGE inline fp32->fp16 cast halves SBUF-write traffic so the
            # HBM/fabric read side binds. Last block chunked for a shorter
            # exposed tail.
            for j in range(1, NBLK):
                src = pred_d[:, j * TBLK:(j + 1) * TBLK, :]
                if j == NBLK - 1:
                    for ci, (c0, c1) in enumerate(QCHUNKS):
                        w = c1 - c0
                        cp = spool.tile([128, QCHMAX], f16, tag="chunkpart")
                        nc.gpsimd.dma_start(out=cp[:, :w],
                                            in_=src[:, :, c0:c1])
                        nc.scalar.activation(
                            cp[:, :w], cp[:, :w], Exp,
                            accum_out=acc[:, j * NQCH + ci:j * NQCH + ci + 1])
                else:
                    ct = spool.tile([128, C], f16, tag="chunk")
                    nc.gpsimd.dma_start(out=ct[:], in_=src)
                    nc.scalar.activation(
                        ct[:], ct[:], Exp,
                        accum_out=acc[:, j * NQCH:j * NQCH + 1])
            nc.sync.dma_start(out=acc_d[:], in_=acc[:])

    nc.compile()
    _CACHE["nc"] = nc
    return nc


def _compose_bands(P, sk):
    """Fuse per-step band matrices into KF-step 9-tap coefficient blocks.

    P: [B, T, S] step probabilities (raw-logit exp, masked states = 0)
    sk: [B, S] skip-transition mask
    Returns Q [B, NFUSED, S, WQ] with Q[..., s, d] = coeff of A_old[s-(GD-d)].
    """
    b1 = P.copy()  # M[s, s-1] coeff, invalid at s=0
    b1[:, :, 0] = 0.0
    b2 = P * sk[:, None, :]  # M[s, s-2] coeff, invalid at s<2
    b2[:, :, :2] = 0.0
    Q = np.zeros((B, NFUSED, S, WQ), dtype=np.float64)
    for tau in range(NFUSED):
        t0 = 1 + tau * KF
        nk = min(KF, T - t0)
        # bands C[o][s] = coeff of A_old[s-o]; start with identity
        Cb = {0: np.ones((B, S), dtype=np.float64)}
        for i in range(nk):
            t = t0 + i
            Mb = {0: P[:, t].astype(np.float64),
                  1: b1[:, t].astype(np.float64),
                  2: b2[:, t].astype(np.float64)}
            Nb = {}
            for o2, m in Mb.items():
                for oc, cvec in Cb.items():
                    o = o2 + oc
                    sh = np.zeros((B, S), dtype=np.float64)
                    sh[:, o2:] = cvec[:, :S - o2] if o2 else cvec
                    term = m * sh
                    if o in Nb:
                        Nb[o] += term
                    else:
                        Nb[o] = term
            Cb = Nb
        for o, cvec in Cb.items():
            Q[:, tau, :, GD - o] = cvec
    return Q


def prepare_in_maps(pred, targets, lens):
    """Host prep: extended labels, gathered probs, fused band coefficients."""
    ext = np.zeros((B, S), dtype=np.int64)
    ext[:, 1::2] = targets
    G = pred[np.arange(B)[:, None, None], np.arange(T)[None, :, None],
             ext[:, None, :]]  # [B, T, S]
    valid = np.arange(S)[None, :] < (2 * lens + 1)[:, None]  # [B, S]
    G = np.where(valid[:, None, :], G, NEG)
    P = np.exp(G.astype(np.float64)).astype(np.float32)  # [B, T, S]
    sk = np.pad((ext[:, 2:] != ext[:, :-2]) & (ext[:, 2:] != 0),
                ((0, 0), (2, 0))).astype(np.float32)  # [B, S]
    Q = _compose_bands(P, sk).astype(np.float32)
    Qb = Q.astype(mybir.dt.np(bf16))
    a0 = np.zeros((B, SG), dtype=np.float32)
    a0[:, GD:GD + 2] = P[:, 0, 0:2]
    in_maps = []
    for c in range(N_CORES):
        sl = slice(c * BS, (c + 1) * BS)
        in_maps.append({
            "pred": np.ascontiguousarray(pred[sl]),
            "q": np.ascontiguousarray(Qb[sl].reshape(BS, NFUSED * QF)),
            "a0": np.ascontiguousarray(a0[sl]),
        })
    return in_maps


def finish_host(results, lens):
    """Combine per-core outputs into the scalar mean loss (float64)."""
    loss_b = np.zeros(B, dtype=np.float64)
    with np.errstate(divide="ignore", invalid="ignore"):
        for c in range(N_CORES):
            r = results[c]
            acc = r["acc"].astype(np.float64)  # [128, NBLK*NQCH]
            lse = np.log(acc.reshape(128, NBLK, NQCH).sum(-1))  # [128, NBLK]
            # row p = b*TBLK + t_off; t = j*TBLK + t_off
            s_lse = lse.reshape(BS, TBLK, NBLK).sum((1, 2))  # [BS]
            afin = r["afin"].astype(np.float64)  # [BS, S]
            rn = r["rnorm"].astype(np.float64)  # [BS, NREN] 1/max multipliers
            log_carry = np.log(rn).sum(1)  # [BS]
            for b in range(BS):
                gb = c * BS + b
                sE = 2 * int(lens[gb])
                le = np.logaddexp(np.log(afin[b, sE]), np.log(afin[b, sE - 1]))
                loss_b[gb] = s_lse[b] + log_carry[b] - le
    loss_b = np.where(loss_b >= 1e29, 0.0, loss_b)
    loss_b = np.where(np.isfinite(loss_b), loss_b, 0.0)
    loss = np.mean(loss_b / np.maximum(lens.astype(np.float64), 1.0))
    return np.float32(loss)


def kernel(pred, targets, targets_lengths):
    pred = np.asarray(pred, dtype=np.float32)
    targets = np.asarray(targets).astype(np.int64)
    lens = np.asarray(targets_lengths).astype(np.int64)

    nc = _build_program()
    in_maps = prepare_in_maps(pred, targets, lens)
    res = run_bass_kernel_spmd(nc, in_maps, core_ids=list(range(N_CORES)))
    return finish_host(res.results, lens)


# revision 10
# speedup vs baseline: 1.2262x; 1.0516x over previous
"""CTC loss kernel for Trainium2 (8 NeuronCores, data-parallel over batch).

Math: with raw logits G[b,t,s] = pred[b,t,ext[b,s]] (ext = blank-interleaved
targets) the CTC forward recursion commutes with the per-frame log-softmax
normalizer: running the recursion on raw logits and subtracting
sum_t logsumexp_c(pred[b,t,:]) at the end gives the same loss. The chip does
(1) sum_c exp(pred) per (b,t) via streaming ACT exp+accumulate (the
memory-bound bulk, ~68 MB/core at the SBUF-fabric ceiling) and (2) the
probability-space forward recursion on the VectorEngine.

The recursion step new[s] = p[s]*(A[s] + A[s-1] + sk[s]*A[s-2]) is linear in
A, so K=4 consecutive steps compose into one 9-tap banded matrix whose
coefficients depend only on p/sk — the host precomputes them (bf16, all
terms positive so errors stay relative). On-chip each fused step is ONE
windowed tensor_mul (overlapping-window AP, free dims [(1,51),(1,9)])
against the coefficient block plus ONE reduce_add: DVE cost follows
(N+151)/0.96ns, so 40 fused steps ≈ 58us of serial chain vs ~110us for
per-step evaluation, fully hidden under the stream. Renormalization (every
8 steps = every 2 fused, against overflow) records the reciprocal of the
running max and folds the multiply into the next fused step's
scalar_tensor_tensor; the host compensates with -log(rn) in float64.
"""

import sys

sys.path.insert(0, "/opt/trn_rl_repo")

import numpy as np

import bass_rust
import concourse.bacc as bacc
import concourse.tile as tile
from concourse import mybir
from concourse.bass_utils import run_bass_kernel_spmd

B, T, C, L = 128, 160, 6625, 25
S = 2 * L + 1  # 51 CTC states
KF = 4  # CTC steps fused per DVE step
WQ = 2 * KF + 1  # 9-tap window
GD = WQ - 1  # 8 guard columns
SG = S + GD  # state tile cols: guards + states
QF = S * WQ  # 459 coefficients per fused step
NSTEP = T - 1  # 159 raw steps
NFUSED = (NSTEP + KF - 1) // KF  # 40 fused steps (last covers 3 raw)
N_CORES = 8
BS = B // N_CORES  # 16 samples per core
TBLK = 8  # t-values per 128-row streaming block (8*16 = 128 rows)
NBLK = T // TBLK  # 20
# finer parts for the first/last streaming block: earlier pipeline start,
# smaller exposed tail.
QCHUNKS = [(0, 1657), (1657, 3313), (3313, 4969), (4969, 6625)]
NQCH = len(QCHUNKS)
QCHMAX = max(c1 - c0 for c0, c1 in QCHUNKS)
NEG = -1.0e4  # exp() underflows to exactly 0.0f
NREN = 19  # renorm after fused steps 1,3,...,37 (raw t = 8,16,...,152)

f32 = mybir.dt.float32
bf16 = mybir.dt.bfloat16
f16 = mybir.dt.float16
Exp = mybir.ActivationFunctionType.Exp

_CACHE = {}


def _win(ap, part_stride, n_part, s_stride):
    """Windowed view [n_part, S, WQ]: addr = offset + s*s_stride + d."""
    v = ap.copy()
    v.ap = bass_rust.VecI64Pair(
        [[part_stride, n_part], [s_stride, S], [1, WQ]])
    return v


def _build_program():
    if "nc" in _CACHE:
        return _CACHE["nc"]
    nc = bacc.Bacc("TRN2", target_bir_lowering=False, debug=False,
                   num_devices=N_CORES)
    pred_d = nc.dram_tensor("pred", [BS, T, C], f32, kind="ExternalInput").ap()
    q_d = nc.dram_tensor("q", [BS, NFUSED * QF], bf16,
                         kind="ExternalInput").ap()
    a0_d = nc.dram_tensor("a0", [BS, SG], f32, kind="ExternalInput").ap()
    acc_d = nc.dram_tensor("acc", [128, NBLK * NQCH], f32,
                           kind="ExternalOutput").ap()
    afin_d = nc.dram_tensor("afin", [BS, S], f32, kind="ExternalOutput").ap()
    rnorm_d = nc.dram_tensor("rnorm", [BS, NREN], f32,
                             kind="ExternalOutput").ap()

    with tile.TileContext(nc) as tc:
        with (
            tc.tile_pool(name="persist", bufs=1) as pp,
            tc.tile_pool(name="steps", bufs=2) as stepp,
            tc.tile_pool(name="stream", bufs=5) as spool,
        ):
            qt = pp.tile([BS, NFUSED * QF], bf16, tag="qt")
            Aa = pp.tile([BS, SG], f32, tag="Aa")
            Ab = pp.tile([BS, SG], f32, tag="Ab")
            Mt = pp.tile([BS, NREN], f32, tag="Mt")
            acc = pp.tile([128, NBLK * NQCH], f32, tag="acc")

            # acc zeroed once; middle blocks only write col j*4+0. Emitted
            # before any ACT accum write so the WAW order is correct.
            nc.vector.memset(acc[:], 0.0)
            nc.vector.memset(Ab[:, 0:GD], 0.0)

            # ---- recursion inputs on the sync ring. They crawl behind the
            # port-saturating SWDGE stream (~0.6 MB total), but the fused
            # recursion chain has ~100us of slack so that's fine. The stream
            # itself runs entirely on SWDGE f16 so the pair's SBUF-write-port
            # budget (the binding resource) carries no f32 traffic at all.
            nc.sync.dma_start(out=Aa[:], in_=a0_d[:])
            qq = (NFUSED * QF) // 4
            for qi in range(4):
                lo = qi * qq
                hi = NFUSED * QF if qi == 3 else lo + qq
                nc.sync.dma_start(out=qt[:, lo:hi], in_=q_d[:, lo:hi])

            # ---- stream block 0, chunked f16 on SWDGE for an early ACT
            # start; ACT has ~15us of slack vs the stream so the SWDGE
            # warmup latency here is harmless.
            for ci, (c0, c1) in enumerate(QCHUNKS):
                w = c1 - c0
                cp = spool.tile([128, QCHMAX], f16, tag="chunkpart")
                nc.gpsimd.dma_start(out=cp[:, :w], in_=pred_d[:, 0:TBLK, c0:c1])
                nc.scalar.activation(cp[:, :w], cp[:, :w], Exp,
                                     accum_out=acc[:, ci:ci + 1])

            # ---- DVE-only fused forward recursion.
            cur, nxt = Aa, Ab
            k = 0
            pend = None  # per-partition scalar to multiply in (renorm fold)
            qstride = NFUSED * QF
            for tau in range(NFUSED):
                wtl = stepp.tile([BS, QF], f32, tag="w")
                av = _win(cur[:], SG, BS, 1)
                qv = _win(qt[:, tau * QF:(tau + 1) * QF], qstride, BS, WQ)
                wv = _win(wtl[:], QF, BS, WQ)
                if pend is None:
                    nc.vector.tensor_mul(out=wv, in0=av, in1=qv)
                else:
                    nc.vector.scalar_tensor_tensor(
                        out=wv, in0=av, scalar=pend, in1=qv,
                        op0=mybir.AluOpType.mult, op1=mybir.AluOpType.mult)
                    pend = None
                nc.vector.tensor_reduce(out=nxt[:, GD:GD + S], in_=wv,
                                        axis=mybir.AxisListType.X,
                                        op=mybir.AluOpType.add)
                if tau % 2 == 1 and k < NREN:
                    mx = stepp.tile([BS, 1], f32, tag="mx")
                    nc.vector.reduce_max(mx[:], nxt[:, GD:GD + S],
                                         axis=mybir.AxisListType.X)
                    # record the actual multiplier; host compensates -log(rn)
                    nc.vector.reciprocal(out=Mt[:, k:k + 1], in_=mx[:])
                    pend = Mt[:, k:k + 1]
                    k += 1
                cur, nxt = nxt, cur
            assert k == NREN
            nc.sync.dma_start(out=afin_d[:], in_=cur[:, GD:GD + S])
            nc.sync.dma_start(out=rnorm_d[:], in_=Mt[:])

            # ---- streaming sum(exp(pred)) over C, 128 (b,t) rows per block.
            # SWDGE inline fp32->fp16 cast halves SBUF-write traffic so the
            # HBM/fabric read side binds. Last block chunked for a shorter
            # exposed tail.
            for j in range(1, NBLK):
                src = pred_d[:, j * TBLK:(j + 1) * TBLK, :]
                if j == NBLK - 1:
                    for ci, (c0, c1) in enumerate(QCHUNKS):
                        w = c1 - c0
                        cp = spool.tile([128, QCHMAX], f16, tag="chunkpart")
                        nc.gpsimd.dma_start(out=cp[:, :w],
                                            in_=src[:, :, c0:c1])
                        nc.scalar.activation(
                            cp[:, :w], cp[:, :w], Exp,
                            accum_out=acc[:, j * NQCH + ci:j * NQCH + ci + 1])
                else:
                    ct = spool.tile([128, C], f16, tag="chunk")
                    nc.gpsimd.dma_start(out=ct[:], in_=src)
                    nc.scalar.activation(
                        ct[:], ct[:], Exp,
                        accum_out=acc[:, j * NQCH:j * NQCH + 1])
            nc.sync.dma_start(out=acc_d[:], in_=acc[:])

    nc.compile()
    _CACHE["nc"] = nc
    return nc


def _compose_bands(P, sk):
    """Fuse per-step band matrices into KF-step 9-tap coefficient blocks.

    P: [B, T, S] step probabilities (raw-logit exp, masked states = 0)
    sk: [B, S] skip-transition mask
    Returns Q [B, NFUSED, S, WQ] with Q[..., s, d] = coeff of A_old[s-(GD-d)].
    """
    b1 = P.copy()  # M[s, s-1] coeff, invalid at s=0
    b1[:, :, 0] = 0.0
    b2 = P * sk[:, None, :]  # M[s, s-2] coeff, invalid at s<2
    b2[:, :, :2] = 0.0
    Q = np.zeros((B, NFUSED, S, WQ), dtype=np.float64)
    for tau in range(NFUSED):
        t0 = 1 + tau * KF
        nk = min(KF, T - t0)
        # bands C[o][s] = coeff of A_old[s-o]; start with identity
        Cb = {0: np.ones((B, S), dtype=np.float64)}
        for i in range(nk):
            t = t0 + i
            Mb = {0: P[:, t].astype(np.float64),
                  1: b1[:, t].astype(np.float64),
                  2: b2[:, t].astype(np.float64)}
            Nb = {}
            for o2, m in Mb.items():
                for oc, cvec in Cb.items():
                    o = o2 + oc
                    sh = np.zeros((B, S), dtype=np.float64)
                    sh[:, o2:] = cvec[:, :S - o2] if o2 else cvec
                    term = m * sh
                    if o in Nb:
                        Nb[o] += term
                    else:
                        Nb[o] = term
            Cb = Nb
        for o, cvec in Cb.items():
            Q[:, tau, :, GD - o] = cvec
    return Q


def prepare_in_maps(pred, targets, lens):
    """Host prep: extended labels, gathered probs, fused band coefficients."""
    ext = np.zeros((B, S), dtype=np.int64)
    ext[:, 1::2] = targets
    G = pred[np.arange(B)[:, None, None], np.arange(T)[None, :, None],
             ext[:, None, :]]  # [B, T, S]
    valid = np.arange(S)[None, :] < (2 * lens + 1)[:, None]  # [B, S]
    G = np.where(valid[:, None, :], G, NEG)
    P = np.exp(G.astype(np.float64)).astype(np.float32)  # [B, T, S]
    sk = np.pad((ext[:, 2:] != ext[:, :-2]) & (ext[:, 2:] != 0),
                ((0, 0), (2, 0))).astype(np.float32)  # [B, S]
    Q = _compose_bands(P, sk).astype(np.float32)
    Qb = Q.astype(mybir.dt.np(bf16))
    a0 = np.zeros((B, SG), dtype=np.float32)
    a0[:, GD:GD + 2] = P[:, 0, 0:2]
    in_maps = []
    for c in range(N_CORES):
        sl = slice(c * BS, (c + 1) * BS)
        in_maps.append({
            "pred": np.ascontiguousarray(pred[sl]),
            "q": np.ascontiguousarray(Qb[sl].reshape(BS, NFUSED * QF)),
            "a0": np.ascontiguousarray(a0[sl]),
        })
    return in_maps


def finish_host(results, lens):
    """Combine per-core outputs into the scalar mean loss (float64)."""
    loss_b = np.zeros(B, dtype=np.float64)
    with np.errstate(divide="ignore", invalid="ignore"):
        for c in range(N_CORES):
            r = results[c]
            acc = r["acc"].astype(np.float64)  # [128, NBLK*NQCH]
            lse = np.log(acc.reshape(128, NBLK, NQCH).sum(-1))  # [128, NBLK]
            # row p = b*TBLK + t_off; t = j*TBLK + t_off
            s_lse = lse.reshape(BS, TBLK, NBLK).sum((1, 2))  # [BS]
            afin = r["afin"].astype(np.float64)  # [BS, S]
            rn = r["rnorm"].astype(np.float64)  # [BS, NREN] 1/max multipliers
            log_carry = np.log(rn).sum(1)  # [BS]
            for b in range(BS):
                gb = c * BS + b
                sE = 2 * int(lens[gb])
                le = np.logaddexp(np.log(afin[b, sE]), np.log(afin[b, sE - 1]))
                loss_b[gb] = s_lse[b] + log_carry[b] - le
    loss_b = np.where(loss_b >= 1e29, 0.0, loss_b)
    loss_b = np.where(np.isfinite(loss_b), loss_b, 0.0)
    loss = np.mean(loss_b / np.maximum(lens.astype(np.float64), 1.0))
    return np.float32(loss)


def kernel(pred, targets, targets_lengths):
    pred = np.asarray(pred, dtype=np.float32)
    targets = np.asarray(targets).astype(np.int64)
    lens = np.asarray(targets_lengths).astype(np.int64)

    nc = _build_program()
    in_maps = prepare_in_maps(pred, targets, lens)
    res = run_bass_kernel_spmd(nc, in_maps, core_ids=list(range(N_CORES)))
    return finish_host(res.results, lens)
